# revision 9
# baseline (speedup 1.0000x reference)
"""Differentiable risk budgeting solve on 8 Trainium2 NeuronCores.

Problem: 20 unrolled iterations of
    Sw   = einsum('bij,bj->bi', sigma, w)
    grad = 2*Sw - beta + lam_s*sign(w) + 2*lam_t*(w - w_prev)
    w    = proj(w - 0.05*grad)          # clip/renorm twice
with B=32768, P=45.

Strategy: pure data parallel over 8 cores (4096 batch rows each).
sigma is cast to fp16 on the host, rows zero-padded 45->46, and kept
entirely SBUF-resident, so HBM traffic is one half-precision pass.

The batched matvec runs as ONE custom DVE instruction per 128-row tile
(RISK_SEG_DOT): a fused multiply + segmented (per-row-reset)
accumulation over the [45,46] tile stream.  The custom op carries a
hand-authored 2x_1P micro-op program (packed fp16 pairs: MUL lo, MUL
hi, pair add, accumulate; accumulator reset at row boundaries via the
SUB_DIM_DONE step state; seed state zeroes the accumulator with
LOGICAL_XOR(acc,acc) so stale pipe state can't leak in) and advertises
perf_max=1, so the DVE streams 2 elements/cycle - measured ~1.75x the
1x fused op and ~2.3x the old multiply+tree.  Accumulation is fp32
inside the pipe; only the written per-pair cums quantize to fp16.
Row dots are the written values at each row's last (zero-pad) column;
GPSIMD extracts them (strided copy) in groups of 4 tiles, overlapped
under the next tiles' bulk, while GPSIMD also computes the
off-critical-path prep term e1 = D - s*sign(w) + cw*w.

The update/projection chain runs once per iteration at full [128,1440]
width in fp32.  Update folded to u = cw*w - 0.1*Sw - s*sign(w) + D
with cw = 1-0.1*lam_t, s = 0.05*lam_s, D = 0.05*beta + 0.1*lam_t*w_prev
(host-folded, lambdas baked as immediates), sign(w>=0) realized
branch-free as min(w*6e4, s).  The reference's +eps inside renorm
shifts results by ~1e-10 relative and is dropped.
"""

import os
import sys

sys.path.insert(0, "/opt/trn_rl_repo")

import numpy as np

import concourse.bacc as bacc
import concourse.bass as bass
import concourse.mybir as mybir
import concourse.tile as tile
from concourse.bass_utils import run_bass_kernel_spmd

N_CORES = 8
B_TOTAL = 32768
P = 45
PJ = 46  # padded row length (even, keeps packed pairs page-aligned)
BC = B_TOTAL // N_CORES  # 4096 batch rows per core
NT = BC // 128  # 32 tiles of 128 rows per core
NSLOT = 4  # rotating full-cum buffers

N_ITER = 20
STEP = 0.05
MAXW = 0.15
BIGH = 60000.0

F32 = mybir.dt.float32
F16 = mybir.dt.float16
ALU = mybir.AluOpType
AX = mybir.AxisListType

# --------------------------------------------------------------------------
# RISK_SEG_DOT: custom DVE op, hand-authored 1x + 2x_1P programs.
# --------------------------------------------------------------------------

_SEG_DOT = None
_PATCHED = False


def _build_uops():
    from concourse.dve_uop import (
        AluInp,
        AluOp as UAlu,
        DelayInp,
        InpSel,
        OutPath,
        OutSel,
        Trigger,
        UopConfig,
        UopDpConfig,
    )

    PD = DelayInp.PREV_DELAY

    def dp(op, a, b, delay_sel=None, delay_en=None):
        d = UopDpConfig(op=op, alu_src0=a, alu_src1=b, alu_out_enable=1)
        if delay_sel is not None:
            d.delay = list(delay_sel)
        if delay_en is not None:
            d.delay_enable = list(delay_en)
        return d

    def tail(dps):
        while len(dps) < 8:
            dps.append(dp(UAlu.BYPASS, AluInp.PREV_ALU_OUT, AluInp.PREV_ALU_OUT))
        return dps

    def uop(inp, inp_en, dps, trigger, next_uop, repeat, req, wr):
        u = UopConfig()
        u.inp = list(inp) + [InpSel.ZERO] * (8 - len(inp))
        u.inp_enable = list(inp_en) + [0] * (8 - len(inp_en))
        u.datapath_config = dps
        u.trigger = tuple(trigger)
        u.next_uop = tuple(next_uop)
        u.repeat_count = repeat
        u.require_inp0, u.require_inp1 = req
        u.out = {p: OutSel.ALU_OUT for p in OutPath}
        u.out_enable = {
            OutPath.WR0_LO: wr[0],
            OutPath.WR0_HI: wr[1],
            OutPath.WR1_LO: 0,
            OutPath.WR1_HI: 0,
        }
        return u

    T = Trigger

    def fsm(inp, inp_en, mk_dp, wr):
        seed = uop(
            inp, inp_en,
            mk_dp(UAlu.LOGICAL_XOR, AluInp.CURR_ALU_OUT, AluInp.CURR_ALU_OUT),
            (T.COUNT, T.NONE, T.NONE), (1, 0, 0), 1, (0, 0), (0, 0),
        )
        steady = uop(
            inp, inp_en,
            mk_dp(UAlu.ADD, AluInp.CURR_ALU_OUT, AluInp.PREV_ALU_OUT),
            (T.SRC_TENSOR_DONE, T.SUB_DIM_DONE, T.NONE), (0, 2, 0), 0,
            (1, 1), wr,
        )
        step = uop(
            inp, inp_en,
            mk_dp(UAlu.BYPASS, AluInp.PREV_ALU_OUT, AluInp.PREV_ALU_OUT),
            (T.SRC_TENSOR_DONE, T.SUB_DIM_DONE, T.COUNT), (0, 2, 1), 1,
            (1, 1), wr,
        )
        return [seed, steady, step]

    # 1x: stage0 product, stage1 accumulator, stage2 scale by CONST_0
    inp1 = [InpSel.ZERO, InpSel.SRC_0, InpSel.SRC_1, InpSel.CONST_0]
    en1 = [0, 1, 1, 1]
    carry = [PD] * 7

    def mk1(acc_op, a, b):
        return tail([
            dp(UAlu.MULTIPLY, AluInp.PREV_DELAY_0, AluInp.PREV_DELAY_1,
               carry, [1, 1, 1, 0, 0, 0, 0]),
            dp(acc_op, a, b, carry, [1, 1, 1, 0, 0, 0, 0]),
            dp(UAlu.MULTIPLY, AluInp.PREV_ALU_OUT, AluInp.PREV_DELAY_2),
        ])

    # 2x: packed pairs; stage0 MUL lo, stage1 MUL hi (capture lo),
    # stage2 pair=lo+hi, stage3 accumulator, stage4 scale by CONST_0;
    # scaled acc -> both 16-bit halves
    inp2 = [InpSel.ZERO, InpSel.SRC_0, InpSel.SRC_1,
            InpSel.SRC_0_HI, InpSel.SRC_1_HI, InpSel.CONST_0]
    en2 = [0, 1, 1, 1, 1, 1]

    def mk2(acc_op, a, b):
        return tail([
            dp(UAlu.MULTIPLY, AluInp.PREV_DELAY_0, AluInp.PREV_DELAY_1,
               carry, [0, 0, 1, 1, 1, 0, 0]),
            dp(UAlu.MULTIPLY, AluInp.PREV_DELAY_2, AluInp.PREV_DELAY_3,
               [DelayInp.PREV_ALU_OUT] + [PD] * 6, [1, 0, 0, 0, 1, 0, 0]),
            dp(UAlu.ADD, AluInp.PREV_ALU_OUT, AluInp.PREV_DELAY_0,
               carry, [0, 0, 0, 0, 1, 0, 0]),
            dp(acc_op, a, b, carry, [0, 0, 0, 0, 1, 0, 0]),
            dp(UAlu.MULTIPLY, AluInp.PREV_ALU_OUT, AluInp.PREV_DELAY_4),
        ])

    return fsm(inp1, en1, mk1, (1, 0)), fsm(inp2, en2, mk2, (1, 1))


def _patch_perf_max():
    global _PATCHED
    if _PATCHED:
        return
    import concourse.bass as bass_mod

    isa_mod = bass_mod.bass_isa
    orig = isa_mod.InstCustomDveAnt

    def patched(*args, **kw):
        if kw.get("op_name") == "RISK_SEG_DOT":
            kw.setdefault("perf_max", 1)
        return orig(*args, **kw)

    isa_mod.InstCustomDveAnt = patched
    _PATCHED = True


def _register_seg_dot():
    global _SEG_DOT
    _patch_perf_max()
    if _SEG_DOT is not None:
        return _SEG_DOT
    import concourse.dve_ops as dve_ops_mod
    from concourse.dve_ops import DveOp, OPS, _COMPILE_CACHE
    from concourse.dve_spec import (
        Spec, Src0, Src1, C0 as SpecC0, scan, AluOp as SAluOp,
    )
    from concourse.dve_uop import DveOpSpec

    name = "RISK_SEG_DOT"
    for op in OPS:
        if op.name == name:
            _SEG_DOT = op
            return op

    def _ref(in0, in1, c0, c1, c2):
        pr = in0.astype(np.float32) * in1.astype(np.float32)
        return np.cumsum(pr, axis=-1, dtype=np.float32) * c0

    sp = Spec(body=scan(SAluOp.ADD, Src0 * Src1) * SpecC0, reference=_ref)
    row = dve_ops_mod._CUSTOM_DVE_ROW_BASE + len(OPS)
    uops_1x, uops_2x = _build_uops()
    spec_obj = DveOpSpec(
        name=name, opcode=row, uops=uops_1x, uops_2x=uops_2x,
        perf_max=1, rd1_en=True,
    )
    spec_obj.validate("v3")
    shas = {v: spec_obj.sha(v) for v in ("v3", "v4")}
    op = DveOp(name, sp, subdim=True, uops_sha=shas)
    OPS.append(op)
    dve_ops_mod._SUB_OPCODE_FOR_NAME[name] = row
    dve_ops_mod.CUSTOM_DVE_SPECS[name] = sp
    for v in ("v3", "v4"):
        _COMPILE_CACHE[(name, v)] = spec_obj
    _SEG_DOT = op
    return op


_ADDCLIP = None
_MULMIN = None


def _register_chain_ops():
    """ADDCLIP: out = min(max(in0+in1, 0), s0); MULMIN: out = min(in0*in1, s0).
    Standard lower()-generated 1x programs, registered at runtime."""
    global _ADDCLIP, _MULMIN
    if _ADDCLIP is not None:
        return _ADDCLIP, _MULMIN
    import concourse.dve_ops as dve_ops_mod
    from concourse.dve_ops import DveOp, OPS
    from concourse.dve_spec import (
        Spec, Src0, Src1, C0, Zero, lower as dve_lower, maxx, minn,
    )
    from concourse.dve_uop import DveOpSpec

    def reg(name, body, ref):
        for op in OPS:
            if op.name == name:
                return op
        sp = Spec(body=body, reference=ref)
        row = dve_ops_mod._CUSTOM_DVE_ROW_BASE + len(OPS)
        shas = {
            v: DveOpSpec(
                name=name, opcode=row, uops=dve_lower(sp, ver=v), rd1_en=True
            ).sha(v)
            for v in ("v3", "v4")
        }
        op = DveOp(name, sp, subdim=False, uops_sha=shas)
        OPS.append(op)
        dve_ops_mod._SUB_OPCODE_FOR_NAME[name] = row
        dve_ops_mod.CUSTOM_DVE_SPECS[name] = sp
        return op

    _ADDCLIP = reg(
        "RISK_ADDCLIP",
        minn(maxx(Src0 + Src1, Zero), C0),
        lambda in0, in1, c0, c1, c2: np.minimum(
            np.maximum(in0.astype(np.float32) + in1, 0.0), c0
        ),
    )
    _MULMIN = reg(
        "RISK_MULMIN",
        minn(Src0 * Src1, C0),
        lambda in0, in1, c0, c1, c2: np.minimum(
            in0.astype(np.float32) * in1, c0
        ),
    )
    return _ADDCLIP, _MULMIN


# --------------------------------------------------------------------------
# Kernel
# --------------------------------------------------------------------------


def _build_program(cw: float, s: float):
    """Trace the per-core Bass program. cw/s are baked as immediates."""
    seg_dot = _register_seg_dot()
    addclip, mulmin = _register_chain_ops()
    nc = bacc.Bacc("TRN2", target_bir_lowering=False, debug=False)

    sig_d = nc.dram_tensor("sigma16", [BC, P * PJ], F16, kind="ExternalInput").ap()
    d_d = nc.dram_tensor("dvec", [BC, P], F32, kind="ExternalInput").ap()
    w_d = nc.dram_tensor("wout", [BC, P], F32, kind="ExternalOutput").ap()

    reps = int(os.environ.get("RISK_KERNEL_BENCH_REPS", "1"))

    import contextlib

    with tile.TileContext(nc) as tc:
        with (
            tc.tile_pool(name="sig", bufs=1) as psig,
            tc.tile_pool(name="wrk", bufs=1) as pwrk,
        ):
            c_cw = pwrk.tile([128, 1], F32, tag="c_cw")
            nc.gpsimd.memset(c_cw[:], cw)

            # ---- resident sigma fp16 (padded rows) + D ----
            sigs = []
            for t in range(NT):
                sig = psig.tile([128, P * PJ], F16, tag=f"sig{t}")
                nc.scalar.dma_start(sig[:], sig_d[t * 128 : (t + 1) * 128])
                sigs.append(sig)
            dt_ = pwrk.tile([128, NT * P], F32, tag="dvec")
            dt3 = dt_[:].rearrange("p (t j) -> p t j", t=NT)
            for t in range(NT):
                nc.scalar.dma_start(dt3[:, t], d_d[t * 128 : (t + 1) * 128])

            # ---- state ----
            w32 = pwrk.tile([128, NT * P], F32, tag="w32")  # compact fp32 w
            nc.gpsimd.memset(w32[:], 1.0 / P)
            w16p = pwrk.tile([128, NT * PJ], F16, tag="w16p")  # padded fp16 w
            nc.gpsimd.memset(w16p[:], 0.0)
            cums = pwrk.tile([128, NSLOT * P * PJ], F16, tag="cums")
            sw16 = pwrk.tile([128, NT * P], F16, tag="sw16")
            e1 = pwrk.tile([128, NT * P], F32, tag="e1")
            e2 = pwrk.tile([128, NT * P], F32, tag="e2")
            u = pwrk.tile([128, NT * P], F32, tag="u")
            r = pwrk.tile([128, NT], F32, tag="r")
            rr = pwrk.tile([128, NT], F32, tag="rr")

            w32_3 = w32[:].rearrange("p (t j) -> p t j", t=NT)
            w16p3 = w16p[:].rearrange("p (t j) -> p t j", j=PJ)
            e1_3 = e1[:].rearrange("p (t j) -> p t j", t=NT)
            e2_3 = e2[:].rearrange("p (t j) -> p t j", t=NT)
            u3 = u[:].rearrange("p (t j) -> p t j", t=NT)
            sw16_3 = sw16[:].rearrange("p (t j) -> p t j", t=NT)
            rr_b = rr[:].unsqueeze(2).broadcast_to([128, NT, P])
            cwb = c_cw[:].unsqueeze(2).broadcast_to([128, NT, P])
            # page-end view of the cum slots: [slot, page] -> elem 45
            GPT = NT // NSLOT  # tile groups per iteration (extractions)
            cums4 = cums[:].rearrange("p (c i j) -> p c i j", c=NSLOT, i=P)
            ends = cums4[:, :, :, P]  # [128, NSLOT, P] strided fp16

            with tc.For_i(0, reps, 1) if reps > 1 else contextlib.nullcontext():
                for it in range(N_ITER):
                    # prep e1 = D - s*sign(w) + cw*w on GPSIMD (under bulk)
                    nc.vector.tensor_scalar(
                        e2[:], w32[:], BIGH, s, ALU.mult, ALU.min
                    )
                    nc.gpsimd.tensor_tensor(e1[:], dt_[:], e2[:], ALU.subtract)
                    nc.gpsimd.tensor_tensor(e2_3, w32_3, cwb, ALU.mult)
                    nc.gpsimd.tensor_tensor(e1[:], e1[:], e2[:], ALU.add)
                    # w16 cast for the fp16 bulk
                    nc.vector.tensor_copy(w16p3[:, :, 0:P], w32_3)

                    # bulk: segmented scan-dot per tile (2x fp16), cum slots
                    # rotate; GPSIMD extracts row dots per group of NSLOT
                    for g in range(GPT):
                        for k in range(NSLOT):
                            t = g * NSLOT + k
                            sig3 = sigs[t][:].rearrange(
                                "p (i j) -> p i j", i=P
                            )
                            w_b = (
                                w16p3[:, t]
                                .unsqueeze(1)
                                .broadcast_to([128, P, PJ])
                            )
                            nc.vector._custom_dve(
                                seg_dot, out=cums4[:, k], in0=sig3, in1=w_b,
                                s0=-2.0 * STEP,
                            )
                        nc.gpsimd.tensor_copy(
                            sw16_3[:, g * NSLOT : (g + 1) * NSLOT], ends
                        )

                    # u = clip(e1 + v, 0, .15) where v = -0.1*Sw (from bulk);
                    # then renorm, clip (fused mul+min), renorm
                    nc.vector._custom_dve(
                        addclip, out=u3, in0=sw16_3, in1=e1_3, s0=MAXW
                    )
                    nc.vector.tensor_reduce(r[:], u3, AX.X, ALU.add)
                    nc.vector.reciprocal(rr[:], r[:])
                    nc.vector._custom_dve(
                        mulmin, out=u3, in0=u3, in1=rr_b, s0=MAXW
                    )
                    nc.vector.tensor_reduce(r[:], u3, AX.X, ALU.add)
                    nc.vector.reciprocal(rr[:], r[:])
                    nc.vector.tensor_tensor(w32_3, u3, rr_b, ALU.mult)

            # ---- store ----
            for t in range(NT):
                nc.scalar.dma_start(w_d[t * 128 : (t + 1) * 128], w32_3[:, t])

    nc.compile()
    return nc


def _fold(beta, w_prev, log_lambda_sparse, log_lambda_turnover):
    lam_s = np.exp(np.float32(log_lambda_sparse), dtype=np.float32)
    lam_t = np.exp(np.float32(log_lambda_turnover), dtype=np.float32)
    cw = float(np.float32(1.0) - np.float32(2 * STEP) * lam_t)
    s = float(np.float32(STEP) * lam_s)
    dvec = (
        np.float32(STEP) * beta + np.float32(2 * STEP) * lam_t * w_prev
    ).astype(np.float32)
    return cw, s, dvec


def make_in_maps(sigma, beta, w_prev, log_lambda_sparse, log_lambda_turnover):
    cw, s, dvec = _fold(beta, w_prev, log_lambda_sparse, log_lambda_turnover)
    sig16p = np.zeros((B_TOTAL, P, PJ), dtype=np.float16)
    sig16p[:, :, :P] = np.asarray(sigma, dtype=np.float32)
    sig16p = sig16p.reshape(B_TOTAL, P * PJ)
    in_maps = []
    for c in range(N_CORES):
        sl = slice(c * BC, (c + 1) * BC)
        in_maps.append({"sigma16": sig16p[sl], "dvec": dvec[sl]})
    return cw, s, in_maps


def kernel(sigma, beta, w_prev, log_lambda_sparse, log_lambda_turnover):
    beta = np.asarray(beta, dtype=np.float32)
    w_prev = np.asarray(w_prev, dtype=np.float32)
    cw, s, in_maps = make_in_maps(
        sigma, beta, w_prev, log_lambda_sparse, log_lambda_turnover
    )
    nc = _build_program(cw, s)
    res = run_bass_kernel_spmd(nc, in_maps, core_ids=list(range(N_CORES)))
    out = np.concatenate([res.results[c]["wout"] for c in range(N_CORES)], axis=0)
    return out.astype(np.float32)


if __name__ == "__main__":
    rng = np.random.default_rng(0)
    A = rng.standard_normal((B_TOTAL, P, P), dtype=np.float32) * 0.1
    sig = np.einsum("bij,bkj->bik", A, A) + 0.1 * np.eye(P, dtype=np.float32)
    bet = rng.random((B_TOTAL, P), dtype=np.float32)
    bet /= bet.sum(-1, keepdims=True)
    wp = np.full((B_TOTAL, P), 1.0 / P, dtype=np.float32)
    out = kernel(
        sigma=sig,
        beta=bet,
        w_prev=wp,
        log_lambda_sparse=np.float32(-3.0),
        log_lambda_turnover=np.float32(-2.0),
    )
    lam_s = np.exp(np.float32(-3.0))
    lam_t = np.exp(np.float32(-2.0))
    n = 256
    w = np.full((n, P), 1.0 / P, dtype=np.float32)
    for _ in range(N_ITER):
        Sw = np.einsum("bij,bj->bi", sig[:n], w)
        g = 2 * Sw - bet[:n] + lam_s * np.sign(w) + 2 * lam_t * (w - wp[:n])
        w = w - STEP * g
        for _ in range(2):
            w = np.clip(w, 0, MAXW)
            w = w / (w.sum(-1, keepdims=True) + 1e-8)
    err = np.abs(out[:n] - w).max() / np.abs(w).max()
    print(out.shape, out.dtype, "absmax-rel vs numpy (256 rows):", err)


# revision 17
# speedup vs baseline: 1.2984x; 1.2984x over previous
"""Differentiable risk budgeting solve on 8 Trainium2 NeuronCores.

Problem: 20 unrolled iterations of
    Sw   = einsum('bij,bj->bi', sigma, w)
    grad = 2*Sw - beta + lam_s*sign(w) + 2*lam_t*(w - w_prev)
    w    = proj(w - 0.05*grad)          # clip/renorm twice
with B=32768, P=45.

Strategy: pure data parallel over 8 cores (4096 batch rows each).
sigma is cast to fp16 on the host, rows zero-padded 45->46, and kept
entirely SBUF-resident, so HBM traffic is one half-precision pass.

The batched matvec runs as ONE custom DVE instruction per 128-row tile
(RISK_SEG_DOT): a fused multiply + segmented (per-row-reset)
accumulation over the [45,46] tile stream.  The custom op carries a
hand-authored 2x_1P micro-op program (packed fp16 pairs: MUL lo, MUL
hi, pair add, accumulate; accumulator reset at row boundaries via the
SUB_DIM_DONE step state; seed state zeroes the accumulator with
LOGICAL_XOR(acc,acc) so stale pipe state can't leak in) and advertises
perf_max=1, so the DVE streams 2 elements/cycle - measured ~1.75x the
1x fused op and ~2.3x the old multiply+tree.  Accumulation is fp32
inside the pipe; only the written per-pair cums quantize to fp16.
Row dots are the written values at each row's last (zero-pad) column;
GPSIMD extracts them (strided copy) every 4 tiles, overlapped under
the next tiles' bulk, while GPSIMD also computes the off-critical-path
prep term e1 = D - s*sign(w) + cw*w.  The cum buffer is 8 slots deep
with 4-tile extraction granularity, so each extraction has 4 tiles of
slack before its slots are reused and the first one queues after
GPSIMD's prep ops without stalling the DVE bulk (A/B-measured faster
than both 4-slot/4-tile and 8-slot/8-tile variants).

The update/projection chain runs once per iteration at full [128,1440]
width in fp32.  Update folded to u = cw*w - 0.1*Sw - s*sign(w) + D
with cw = 1-0.1*lam_t, s = 0.05*lam_s, D = 0.05*beta + 0.1*lam_t*w_prev
(host-folded, lambdas baked as immediates), sign(w>=0) realized
branch-free as min(w*6e4, s).  The reference's +eps inside renorm
shifts results by ~1e-10 relative and is dropped.
"""

import os
import sys

sys.path.insert(0, "/opt/trn_rl_repo")

import numpy as np

import concourse.bacc as bacc
import concourse.bass as bass
import concourse.mybir as mybir
import concourse.tile as tile
from concourse.bass_utils import run_bass_kernel_spmd

N_CORES = 8
B_TOTAL = 32768
P = 45
PJ = 46  # padded row length (even, keeps packed pairs page-aligned)
BC = B_TOTAL // N_CORES  # 4096 batch rows per core
NT = BC // 128  # 32 tiles of 128 rows per core
NSLOT = 8  # rotating full-cum buffers (override: RISK_NSLOT)

N_ITER = 20
STEP = 0.05
MAXW = 0.15
BIGH = 60000.0

F32 = mybir.dt.float32
F16 = mybir.dt.float16
ALU = mybir.AluOpType
AX = mybir.AxisListType

# --------------------------------------------------------------------------
# RISK_SEG_DOT: custom DVE op, hand-authored 1x + 2x_1P programs.
# --------------------------------------------------------------------------

_SEG_DOT = None
_PATCHED = False


def _build_uops():
    from concourse.dve_uop import (
        AluInp,
        AluOp as UAlu,
        DelayInp,
        InpSel,
        OutPath,
        OutSel,
        Trigger,
        UopConfig,
        UopDpConfig,
    )

    PD = DelayInp.PREV_DELAY

    def dp(op, a, b, delay_sel=None, delay_en=None):
        d = UopDpConfig(op=op, alu_src0=a, alu_src1=b, alu_out_enable=1)
        if delay_sel is not None:
            d.delay = list(delay_sel)
        if delay_en is not None:
            d.delay_enable = list(delay_en)
        return d

    def tail(dps):
        while len(dps) < 8:
            dps.append(dp(UAlu.BYPASS, AluInp.PREV_ALU_OUT, AluInp.PREV_ALU_OUT))
        return dps

    def uop(inp, inp_en, dps, trigger, next_uop, repeat, req, wr):
        u = UopConfig()
        u.inp = list(inp) + [InpSel.ZERO] * (8 - len(inp))
        u.inp_enable = list(inp_en) + [0] * (8 - len(inp_en))
        u.datapath_config = dps
        u.trigger = tuple(trigger)
        u.next_uop = tuple(next_uop)
        u.repeat_count = repeat
        u.require_inp0, u.require_inp1 = req
        u.out = {p: OutSel.ALU_OUT for p in OutPath}
        u.out_enable = {
            OutPath.WR0_LO: wr[0],
            OutPath.WR0_HI: wr[1],
            OutPath.WR1_LO: 0,
            OutPath.WR1_HI: 0,
        }
        return u

    T = Trigger

    def fsm(inp, inp_en, mk_dp, wr):
        seed = uop(
            inp, inp_en,
            mk_dp(UAlu.LOGICAL_XOR, AluInp.CURR_ALU_OUT, AluInp.CURR_ALU_OUT),
            (T.COUNT, T.NONE, T.NONE), (1, 0, 0), 1, (0, 0), (0, 0),
        )
        steady = uop(
            inp, inp_en,
            mk_dp(UAlu.ADD, AluInp.CURR_ALU_OUT, AluInp.PREV_ALU_OUT),
            (T.SRC_TENSOR_DONE, T.SUB_DIM_DONE, T.NONE), (0, 2, 0), 0,
            (1, 1), wr,
        )
        step = uop(
            inp, inp_en,
            mk_dp(UAlu.BYPASS, AluInp.PREV_ALU_OUT, AluInp.PREV_ALU_OUT),
            (T.SRC_TENSOR_DONE, T.SUB_DIM_DONE, T.COUNT), (0, 2, 1), 1,
            (1, 1), wr,
        )
        return [seed, steady, step]

    # 1x: stage0 product, stage1 accumulator, stage2 scale by CONST_0
    inp1 = [InpSel.ZERO, InpSel.SRC_0, InpSel.SRC_1, InpSel.CONST_0]
    en1 = [0, 1, 1, 1]
    carry = [PD] * 7

    def mk1(acc_op, a, b):
        return tail([
            dp(UAlu.MULTIPLY, AluInp.PREV_DELAY_0, AluInp.PREV_DELAY_1,
               carry, [1, 1, 1, 0, 0, 0, 0]),
            dp(acc_op, a, b, carry, [1, 1, 1, 0, 0, 0, 0]),
            dp(UAlu.MULTIPLY, AluInp.PREV_ALU_OUT, AluInp.PREV_DELAY_2),
        ])

    # 2x: packed pairs; stage0 MUL lo, stage1 MUL hi (capture lo),
    # stage2 pair=lo+hi, stage3 accumulator, stage4 scale by CONST_0;
    # scaled acc -> both 16-bit halves
    inp2 = [InpSel.ZERO, InpSel.SRC_0, InpSel.SRC_1,
            InpSel.SRC_0_HI, InpSel.SRC_1_HI, InpSel.CONST_0]
    en2 = [0, 1, 1, 1, 1, 1]

    def mk2(acc_op, a, b):
        return tail([
            dp(UAlu.MULTIPLY, AluInp.PREV_DELAY_0, AluInp.PREV_DELAY_1,
               carry, [0, 0, 1, 1, 1, 0, 0]),
            dp(UAlu.MULTIPLY, AluInp.PREV_DELAY_2, AluInp.PREV_DELAY_3,
               [DelayInp.PREV_ALU_OUT] + [PD] * 6, [1, 0, 0, 0, 1, 0, 0]),
            dp(UAlu.ADD, AluInp.PREV_ALU_OUT, AluInp.PREV_DELAY_0,
               carry, [0, 0, 0, 0, 1, 0, 0]),
            dp(acc_op, a, b, carry, [0, 0, 0, 0, 1, 0, 0]),
            dp(UAlu.MULTIPLY, AluInp.PREV_ALU_OUT, AluInp.PREV_DELAY_4),
        ])

    return fsm(inp1, en1, mk1, (1, 0)), fsm(inp2, en2, mk2, (1, 1))


def _patch_perf_max():
    global _PATCHED
    if _PATCHED:
        return
    import concourse.bass as bass_mod

    isa_mod = bass_mod.bass_isa
    orig = isa_mod.InstCustomDveAnt

    def patched(*args, **kw):
        if kw.get("op_name") == "RISK_SEG_DOT":
            kw.setdefault("perf_max", 1)
        return orig(*args, **kw)

    isa_mod.InstCustomDveAnt = patched
    _PATCHED = True


def _register_seg_dot():
    global _SEG_DOT
    _patch_perf_max()
    if _SEG_DOT is not None:
        return _SEG_DOT
    import concourse.dve_ops as dve_ops_mod
    from concourse.dve_ops import DveOp, OPS, _COMPILE_CACHE
    from concourse.dve_spec import (
        Spec, Src0, Src1, C0 as SpecC0, scan, AluOp as SAluOp,
    )
    from concourse.dve_uop import DveOpSpec

    name = "RISK_SEG_DOT"
    for op in OPS:
        if op.name == name:
            _SEG_DOT = op
            return op

    def _ref(in0, in1, c0, c1, c2):
        pr = in0.astype(np.float32) * in1.astype(np.float32)
        return np.cumsum(pr, axis=-1, dtype=np.float32) * c0

    sp = Spec(body=scan(SAluOp.ADD, Src0 * Src1) * SpecC0, reference=_ref)
    row = dve_ops_mod._CUSTOM_DVE_ROW_BASE + len(OPS)
    uops_1x, uops_2x = _build_uops()
    spec_obj = DveOpSpec(
        name=name, opcode=row, uops=uops_1x, uops_2x=uops_2x,
        perf_max=1, rd1_en=True,
    )
    spec_obj.validate("v3")
    shas = {v: spec_obj.sha(v) for v in ("v3", "v4")}
    op = DveOp(name, sp, subdim=True, uops_sha=shas)
    OPS.append(op)
    dve_ops_mod._SUB_OPCODE_FOR_NAME[name] = row
    dve_ops_mod.CUSTOM_DVE_SPECS[name] = sp
    for v in ("v3", "v4"):
        _COMPILE_CACHE[(name, v)] = spec_obj
    _SEG_DOT = op
    return op


_ADDCLIP = None
_MULMIN = None


def _register_chain_ops():
    """ADDCLIP: out = min(max(in0+in1, 0), s0); MULMIN: out = min(in0*in1, s0).
    Standard lower()-generated 1x programs, registered at runtime."""
    global _ADDCLIP, _MULMIN
    if _ADDCLIP is not None:
        return _ADDCLIP, _MULMIN
    import concourse.dve_ops as dve_ops_mod
    from concourse.dve_ops import DveOp, OPS
    from concourse.dve_spec import (
        Spec, Src0, Src1, C0, Zero, lower as dve_lower, maxx, minn,
    )
    from concourse.dve_uop import DveOpSpec

    def reg(name, body, ref):
        for op in OPS:
            if op.name == name:
                return op
        sp = Spec(body=body, reference=ref)
        row = dve_ops_mod._CUSTOM_DVE_ROW_BASE + len(OPS)
        shas = {
            v: DveOpSpec(
                name=name, opcode=row, uops=dve_lower(sp, ver=v), rd1_en=True
            ).sha(v)
            for v in ("v3", "v4")
        }
        op = DveOp(name, sp, subdim=False, uops_sha=shas)
        OPS.append(op)
        dve_ops_mod._SUB_OPCODE_FOR_NAME[name] = row
        dve_ops_mod.CUSTOM_DVE_SPECS[name] = sp
        return op

    _ADDCLIP = reg(
        "RISK_ADDCLIP",
        minn(maxx(Src0 + Src1, Zero), C0),
        lambda in0, in1, c0, c1, c2: np.minimum(
            np.maximum(in0.astype(np.float32) + in1, 0.0), c0
        ),
    )
    _MULMIN = reg(
        "RISK_MULMIN",
        minn(Src0 * Src1, C0),
        lambda in0, in1, c0, c1, c2: np.minimum(
            in0.astype(np.float32) * in1, c0
        ),
    )
    return _ADDCLIP, _MULMIN


# --------------------------------------------------------------------------
# Kernel
# --------------------------------------------------------------------------


def _build_program(cw: float, s: float):
    """Trace the per-core Bass program. cw/s are baked as immediates."""
    seg_dot = _register_seg_dot()
    addclip, mulmin = _register_chain_ops()
    NSLOT = int(os.environ.get("RISK_NSLOT", "8"))
    nc = bacc.Bacc("TRN2", target_bir_lowering=False, debug=False)

    sig_d = nc.dram_tensor("sigma16", [BC, P * PJ], F16, kind="ExternalInput").ap()
    d_d = nc.dram_tensor("dvec", [BC, P], F32, kind="ExternalInput").ap()
    w_d = nc.dram_tensor("wout", [BC, P], F32, kind="ExternalOutput").ap()

    reps = int(os.environ.get("RISK_KERNEL_BENCH_REPS", "1"))

    import contextlib

    with tile.TileContext(nc) as tc:
        with (
            tc.tile_pool(name="sig", bufs=1) as psig,
            tc.tile_pool(name="wrk", bufs=1) as pwrk,
        ):
            c_cw = pwrk.tile([128, 1], F32, tag="c_cw")
            nc.gpsimd.memset(c_cw[:], cw)

            # ---- resident sigma fp16 (padded rows) + D ----
            sigs = []
            for t in range(NT):
                sig = psig.tile([128, P * PJ], F16, tag=f"sig{t}")
                nc.scalar.dma_start(sig[:], sig_d[t * 128 : (t + 1) * 128])
                sigs.append(sig)
            dt_ = pwrk.tile([128, NT * P], F32, tag="dvec")
            dt3 = dt_[:].rearrange("p (t j) -> p t j", t=NT)
            for t in range(NT):
                nc.scalar.dma_start(dt3[:, t], d_d[t * 128 : (t + 1) * 128])

            # ---- state ----
            w32 = pwrk.tile([128, NT * P], F32, tag="w32")  # compact fp32 w
            nc.gpsimd.memset(w32[:], 1.0 / P)
            w16p = pwrk.tile([128, NT * PJ], F16, tag="w16p")  # padded fp16 w
            nc.gpsimd.memset(w16p[:], 0.0)
            cums = pwrk.tile([128, NSLOT * P * PJ], F16, tag="cums")
            sw16 = pwrk.tile([128, NT * P], F16, tag="sw16")
            e1 = pwrk.tile([128, NT * P], F32, tag="e1")
            e2 = pwrk.tile([128, NT * P], F32, tag="e2")
            u = e2  # reused: prep consumes e2 before the chain writes u
            r = pwrk.tile([128, NT], F32, tag="r")
            rr = pwrk.tile([128, NT], F32, tag="rr")

            w32_3 = w32[:].rearrange("p (t j) -> p t j", t=NT)
            w16p3 = w16p[:].rearrange("p (t j) -> p t j", j=PJ)
            e1_3 = e1[:].rearrange("p (t j) -> p t j", t=NT)
            e2_3 = e2[:].rearrange("p (t j) -> p t j", t=NT)
            u3 = u[:].rearrange("p (t j) -> p t j", t=NT)
            sw16_3 = sw16[:].rearrange("p (t j) -> p t j", t=NT)
            rr_b = rr[:].unsqueeze(2).broadcast_to([128, NT, P])
            cwb = c_cw[:].unsqueeze(2).broadcast_to([128, NT, P])
            # page-end view of the cum slots: [slot, page] -> elem 45
            GPT = NT // NSLOT  # tile groups per iteration (extractions)
            cums4 = cums[:].rearrange("p (c i j) -> p c i j", c=NSLOT, i=P)
            ends = cums4[:, :, :, P]  # [128, NSLOT, P] strided fp16

            with tc.For_i(0, reps, 1) if reps > 1 else contextlib.nullcontext():
                for it in range(N_ITER):
                    # prep e1 = D - s*sign(w) + cw*w on GPSIMD (under bulk)
                    nc.vector.tensor_scalar(
                        e2[:], w32[:], BIGH, s, ALU.mult, ALU.min
                    )
                    nc.gpsimd.tensor_tensor(e1[:], dt_[:], e2[:], ALU.subtract)
                    nc.gpsimd.tensor_tensor(e2_3, w32_3, cwb, ALU.mult)
                    nc.gpsimd.tensor_tensor(e1[:], e1[:], e2[:], ALU.add)
                    # w16 cast for the fp16 bulk (Act engine frees ~1.6us
                    # of serial DVE time per iteration; RISK_CAST=dve reverts)
                    if os.environ.get("RISK_CAST", "act") == "act":
                        nc.scalar.copy(w16p3[:, :, 0:P], w32_3)
                    else:
                        nc.vector.tensor_copy(w16p3[:, :, 0:P], w32_3)

                    # bulk: segmented scan-dot per tile (2x fp16), cum slots
                    # rotate (NSLOT deep); GPSIMD extracts row dots every
                    # EXG tiles, so each extraction has NSLOT-EXG tiles of
                    # slack before its slots are reused and the first one
                    # queues after GPSIMD's prep without stalling the DVE
                    EXG = min(int(os.environ.get("RISK_EXG", "4")), NSLOT)
                    for t in range(NT):
                        sig3 = sigs[t][:].rearrange("p (i j) -> p i j", i=P)
                        w_b = (
                            w16p3[:, t]
                            .unsqueeze(1)
                            .broadcast_to([128, P, PJ])
                        )
                        nc.vector._custom_dve(
                            seg_dot, out=cums4[:, t % NSLOT], in0=sig3,
                            in1=w_b, s0=-2.0 * STEP,
                        )
                        if t % EXG == EXG - 1:
                            k0 = (t - EXG + 1) % NSLOT
                            nc.gpsimd.tensor_copy(
                                sw16_3[:, t - EXG + 1 : t + 1],
                                ends[:, k0 : k0 + EXG],
                            )

                    # u = clip(e1 + v, 0, .15) where v = -0.1*Sw (from bulk);
                    # then renorm, clip (fused mul+min), renorm
                    CHAIN = os.environ.get("RISK_CHAIN", "fused")
                    if CHAIN == "fused":
                        nc.vector._custom_dve(
                            addclip, out=u3, in0=sw16_3, in1=e1_3, s0=MAXW
                        )
                    else:
                        nc.vector.tensor_tensor(u3, sw16_3, e1_3, ALU.add)
                        nc.vector.tensor_scalar(
                            u[:], u[:], 0.0, MAXW, ALU.max, ALU.min
                        )
                    nc.vector.tensor_reduce(r[:], u3, AX.X, ALU.add)
                    nc.vector.reciprocal_approx_fast(rr[:], r[:])
                    if CHAIN == "fused":
                        nc.vector._custom_dve(
                            mulmin, out=u3, in0=u3, in1=rr_b, s0=MAXW
                        )
                    else:
                        nc.vector.tensor_tensor(u3, u3, rr_b, ALU.mult)
                        nc.vector.tensor_scalar(
                            u[:], u[:], MAXW, None, ALU.min
                        )
                    nc.vector.tensor_reduce(r[:], u3, AX.X, ALU.add)
                    nc.vector.reciprocal_approx_fast(rr[:], r[:])
                    nc.vector.tensor_tensor(w32_3, u3, rr_b, ALU.mult)

            # ---- store ----
            for t in range(NT):
                nc.scalar.dma_start(w_d[t * 128 : (t + 1) * 128], w32_3[:, t])

    nc.compile()
    return nc


def _fold(beta, w_prev, log_lambda_sparse, log_lambda_turnover):
    lam_s = np.exp(np.float32(log_lambda_sparse), dtype=np.float32)
    lam_t = np.exp(np.float32(log_lambda_turnover), dtype=np.float32)
    cw = float(np.float32(1.0) - np.float32(2 * STEP) * lam_t)
    s = float(np.float32(STEP) * lam_s)
    dvec = (
        np.float32(STEP) * beta + np.float32(2 * STEP) * lam_t * w_prev
    ).astype(np.float32)
    return cw, s, dvec


def make_in_maps(sigma, beta, w_prev, log_lambda_sparse, log_lambda_turnover):
    cw, s, dvec = _fold(beta, w_prev, log_lambda_sparse, log_lambda_turnover)
    sig16p = np.zeros((B_TOTAL, P, PJ), dtype=np.float16)
    sig16p[:, :, :P] = np.asarray(sigma, dtype=np.float32)
    sig16p = sig16p.reshape(B_TOTAL, P * PJ)
    in_maps = []
    for c in range(N_CORES):
        sl = slice(c * BC, (c + 1) * BC)
        in_maps.append({"sigma16": sig16p[sl], "dvec": dvec[sl]})
    return cw, s, in_maps


def kernel(sigma, beta, w_prev, log_lambda_sparse, log_lambda_turnover):
    beta = np.asarray(beta, dtype=np.float32)
    w_prev = np.asarray(w_prev, dtype=np.float32)
    cw, s, in_maps = make_in_maps(
        sigma, beta, w_prev, log_lambda_sparse, log_lambda_turnover
    )
    nc = _build_program(cw, s)
    res = run_bass_kernel_spmd(nc, in_maps, core_ids=list(range(N_CORES)))
    out = np.concatenate([res.results[c]["wout"] for c in range(N_CORES)], axis=0)
    return out.astype(np.float32)


if __name__ == "__main__":
    rng = np.random.default_rng(0)
    A = rng.standard_normal((B_TOTAL, P, P), dtype=np.float32) * 0.1
    sig = np.einsum("bij,bkj->bik", A, A) + 0.1 * np.eye(P, dtype=np.float32)
    bet = rng.random((B_TOTAL, P), dtype=np.float32)
    bet /= bet.sum(-1, keepdims=True)
    wp = np.full((B_TOTAL, P), 1.0 / P, dtype=np.float32)
    out = kernel(
        sigma=sig,
        beta=bet,
        w_prev=wp,
        log_lambda_sparse=np.float32(-3.0),
        log_lambda_turnover=np.float32(-2.0),
    )
    lam_s = np.exp(np.float32(-3.0))
    lam_t = np.exp(np.float32(-2.0))
    n = 256
    w = np.full((n, P), 1.0 / P, dtype=np.float32)
    for _ in range(N_ITER):
        Sw = np.einsum("bij,bj->bi", sig[:n], w)
        g = 2 * Sw - bet[:n] + lam_s * np.sign(w) + 2 * lam_t * (w - wp[:n])
        w = w - STEP * g
        for _ in range(2):
            w = np.clip(w, 0, MAXW)
            w = w / (w.sum(-1, keepdims=True) + 1e-8)
    err = np.abs(out[:n] - w).max() / np.abs(w).max()
    print(out.shape, out.dtype, "absmax-rel vs numpy (256 rows):", err)
